# revision 6
# baseline (speedup 1.0000x reference)
"""MoE top-2 routing kernel for Trainium2, expert-parallel over 8 NeuronCores.

Problem (fp32):
  x [2, 2048, 512], gate Wg [512, 8] (+bg), experts W1 [8, 512, 2048] (+b1),
  W2 [8, 2048, 512] (+b2), top-2 softmax-renormalized combine.

Sharding: expert parallelism — core e holds expert e's weights. The gate is
replicated (every core computes the full gate for all 4096 tokens, then keeps
only its own expert's combine column). Each core computes y_e = combine[:, e]
* FFN_e(x) for the tokens it owns and the host unshard step sums the 8
partial outputs.

Matmul precision: the gate matmul runs in full fp32 (so top-2 selection
matches the fp32 reference; a routing flip would be a large error), the FFN
matmuls run as float32r (full PE rate at free-dim >= 256).
"""

import os
import sys

import numpy as np

for _p in ("/opt/trn_rl_repo",):
    if _p not in sys.path and os.path.isdir(_p):
        sys.path.insert(0, _p)

P = 128          # SBUF partitions
D = 512          # d_model
F = 2048         # d_ff
T = 4096         # tokens (B*S)
E = 8            # experts
KD = D // P      # 4  k-tiles over d_model
KF = F // P      # 16 f-tiles over d_ff
NT = T // P      # 32 token tiles
CH = 512         # token chunk width for hT matmuls (>=256 keeps f32r at rate)
NCH = T // CH    # 8 chunks
NCORES = 8

_CACHE = {}


def _build_dense():
    """One SPMD program: dense per-expert FFN over all tokens."""
    from contextlib import ExitStack

    import concourse.bass as bass
    import concourse.mybir as mybir
    import concourse.tile as tile
    from concourse import bacc

    f32 = mybir.dt.float32
    f32r = mybir.dt.float32r
    ts = bass.ts

    nc = bacc.Bacc("TRN2", target_bir_lowering=False)
    xT = nc.declare_dram_parameter("xT", [D, T], f32, isOutput=False)
    wg = nc.declare_dram_parameter("Wg", [D, E], f32, isOutput=False)
    bgbc = nc.declare_dram_parameter("bgbc", [P, E], f32, isOutput=False)
    w1 = nc.declare_dram_parameter("W1e", [D, F], f32r, isOutput=False)
    b1p = nc.declare_dram_parameter("b1p", [P, KF], f32, isOutput=False)
    w2 = nc.declare_dram_parameter("W2e", [F, D], f32r, isOutput=False)
    b2bc = nc.declare_dram_parameter("b2bc", [P, D], f32, isOutput=False)
    esel = nc.declare_dram_parameter("esel", [P, E], f32, isOutput=False)
    y = nc.declare_dram_parameter("y", [T, D], f32, isOutput=True)

    with tile.TileContext(nc) as tc, ExitStack() as ctx:
        consts = ctx.enter_context(tc.tile_pool(name="consts", bufs=1))
        gate_ps = ctx.enter_context(tc.tile_pool(name="gate_ps", bufs=2, space="PSUM"))
        gate_sb = ctx.enter_context(tc.tile_pool(name="gate_sb", bufs=4))
        h_ps = ctx.enter_context(tc.tile_pool(name="h_ps", bufs=3, space="PSUM"))
        h_pool = ctx.enter_context(tc.tile_pool(name="h_pool", bufs=18))
        y_ps = ctx.enter_context(tc.tile_pool(name="y_ps", bufs=2, space="PSUM"))
        y_sb = ctx.enter_context(tc.tile_pool(name="y_sb", bufs=3))
        xtr_pool = ctx.enter_context(tc.tile_pool(name="xtr_pool", bufs=2))

        # ---- resident loads ----
        wg_t = [consts.tile([P, E], f32, name=f"wg{k}", tag=f"wg{k}") for k in range(KD)]
        for k in range(KD):
            nc.sync.dma_start(wg_t[k][:], wg[ts(k, P), :])
        bgbc_t = consts.tile([P, E], f32, tag="bgbc")
        nc.sync.dma_start(bgbc_t[:], bgbc[:, :])
        esel_t = consts.tile([P, E], f32, tag="esel")
        nc.sync.dma_start(esel_t[:], esel[:, :])

        xt_t = [consts.tile([P, T], f32, name=f"xt{k}", tag=f"xt{k}") for k in range(KD)]
        # chunked loads so the gate can start before the whole tile lands
        for k in range(KD):
            for h in range(4):
                nc.sync.dma_start(xt_t[k][:, ts(h, T // 4)], xT[ts(k, P), ts(h, T // 4)])

        w1_t = [consts.tile([P, F], f32r, name=f"w1{k}", tag=f"w1{k}") for k in range(KD)]
        for k in range(KD):
            nc.sync.dma_start(w1_t[k][:], w1[ts(k, P), :])
        w2_t = [consts.tile([P, D], f32r, name=f"w2{f}", tag=f"w2{f}") for f in range(KF)]
        for f in range(KF):
            nc.sync.dma_start(w2_t[f][:], w2[ts(f, P), :])
        b1p_t = consts.tile([P, KF], f32, tag="b1p")
        nc.sync.dma_start(b1p_t[:], b1p[:, :])
        b2bc_t = consts.tile([P, D], f32, tag="b2bc")
        nc.sync.dma_start(b2bc_t[:], b2bc[:, :])

        cvec = consts.tile([P, NT], f32, tag="cvec")  # combine[:, e] per token

        # ---- gate: logits -> exp -> top2 renormalized combine column ----
        for tt in range(NT):
            gps = gate_ps.tile([P, E], f32)
            for k in range(KD):
                nc.tensor.matmul(
                    gps[:],
                    lhsT=xt_t[k][:, ts(tt, P)],
                    rhs=wg_t[k][:],
                    start=(k == 0),
                    stop=(k == KD - 1),
                )
            glog = gate_sb.tile([P, E], f32, tag="glog")
            nc.vector.tensor_tensor(
                out=glog[:], in0=gps[:], in1=bgbc_t[:], op=mybir.AluOpType.add
            )
            probs = gate_sb.tile([P, E], f32, tag="probs")
            nc.scalar.activation(probs[:], glog[:], mybir.ActivationFunctionType.Exp)
            m8 = gate_sb.tile([P, 8], f32, tag="m8")
            nc.vector.max(out=m8[:], in_=probs[:])
            den = gate_sb.tile([P, 1], f32, tag="den")
            nc.vector.tensor_tensor(
                out=den[:], in0=m8[:, 0:1], in1=m8[:, 1:2], op=mybir.AluOpType.add
            )
            rcp = gate_sb.tile([P, 1], f32, tag="rcp")
            nc.vector.reciprocal(rcp[:], den[:])
            # mask of top-2 positions (probs >= second max)
            mask = gate_sb.tile([P, E], f32, tag="mask")
            nc.vector.tensor_scalar(
                out=mask[:],
                in0=probs[:],
                scalar1=m8[:, 1:2],
                scalar2=None,
                op0=mybir.AluOpType.is_ge,
            )
            pm = gate_sb.tile([P, E], f32, tag="pm")
            nc.vector.tensor_tensor(
                out=pm[:], in0=probs[:], in1=mask[:], op=mybir.AluOpType.mult
            )
            pe_ = gate_sb.tile([P, E], f32, tag="pe_")
            nc.vector.tensor_tensor(
                out=pe_[:], in0=pm[:], in1=esel_t[:], op=mybir.AluOpType.mult
            )
            csum = gate_sb.tile([P, 1], f32, tag="csum")
            nc.vector.tensor_reduce(
                out=csum[:],
                in_=pe_[:],
                axis=mybir.AxisListType.X,
                op=mybir.AluOpType.add,
            )
            nc.vector.tensor_scalar_mul(cvec[:, tt : tt + 1], csum[:], rcp[:])

        # ---- FFN: hT = relu(W1.T x + b1); y = hT.T W2 + b2; y *= combine ----
        for cc in range(NCH):
            xtr = []
            for k in range(KD):
                xr = xtr_pool.tile([P, CH], f32r, name=f"xr{k}", tag=f"xr{k}")
                nc.vector.tensor_copy(xr[:], xt_t[k][:, ts(cc, CH)])
                xtr.append(xr)
            hts = []
            for ft in range(KF):
                hp = h_ps.tile([P, CH], f32)
                for k in range(KD):
                    nc.tensor.matmul(
                        hp[:],
                        lhsT=w1_t[k][:, ts(ft, P)],
                        rhs=xtr[k][:],
                        start=(k == 0),
                        stop=(k == KD - 1),
                    )
                hs = h_pool.tile([P, CH], f32r, tag="hs")
                nc.scalar.activation(
                    hs[:],
                    hp[:],
                    mybir.ActivationFunctionType.Relu,
                    bias=b1p_t[:, ft : ft + 1],
                )
                hts.append(hs)
            for st in range(CH // P):
                tt = cc * (CH // P) + st
                yp = y_ps.tile([P, D], f32)
                for fk in range(KF):
                    nc.tensor.matmul(
                        yp[:],
                        lhsT=hts[fk][:, ts(st, P)],
                        rhs=w2_t[fk][:],
                        start=(fk == 0),
                        stop=(fk == KF - 1),
                    )
                ys = y_sb.tile([P, D], f32, tag="ys")
                nc.vector.tensor_tensor(
                    out=ys[:], in0=yp[:], in1=b2bc_t[:], op=mybir.AluOpType.add
                )
                nc.vector.tensor_scalar_mul(ys[:], ys[:], cvec[:, tt : tt + 1])
                nc.sync.dma_start(y[ts(tt, P), :], ys[:])

    nc.compile()
    return nc


def _get_program():
    if "nc" not in _CACHE:
        _CACHE["nc"] = _build_dense()
    return _CACHE["nc"]


def _make_in_maps(x, Wg, bg, W1, b1, W2, b2):
    x = np.ascontiguousarray(np.asarray(x, dtype=np.float32).reshape(T, D))
    Wg = np.ascontiguousarray(np.asarray(Wg, dtype=np.float32))
    bg = np.asarray(bg, dtype=np.float32)
    W1 = np.asarray(W1, dtype=np.float32)
    b1 = np.asarray(b1, dtype=np.float32)
    W2 = np.asarray(W2, dtype=np.float32)
    b2 = np.asarray(b2, dtype=np.float32)

    xt = np.ascontiguousarray(x.T)  # [D, T]
    bgbc = np.ascontiguousarray(np.broadcast_to(bg, (P, E)))

    in_maps = []
    for e in range(NCORES):
        esel = np.zeros((P, E), dtype=np.float32)
        esel[:, e] = 1.0
        in_maps.append(
            {
                "xT": xt,
                "Wg": Wg,
                "bgbc": bgbc,
                "W1e": np.ascontiguousarray(W1[e]),
                "b1p": np.ascontiguousarray(b1[e].reshape(KF, P).T),
                "W2e": np.ascontiguousarray(W2[e]),
                "b2bc": np.ascontiguousarray(np.broadcast_to(b2[e], (P, D))),
                "esel": esel,
            }
        )
    return in_maps


def run(inputs, trace=False):
    """Run the kernel; returns (out [2,2048,512] f32, exec_time_ns or None)."""
    from concourse.bass_utils import run_bass_kernel_spmd

    nc = _get_program()
    in_maps = _make_in_maps(**inputs)
    res = run_bass_kernel_spmd(nc, in_maps, list(range(NCORES)), trace=trace)
    acc = np.zeros((T, D), dtype=np.float32)
    for r in res.results:
        acc += np.asarray(r["y"], dtype=np.float32).reshape(T, D)
    return acc.reshape(2, 2048, D), res.exec_time_ns


def kernel(x, Wg, bg, W1, b1, W2, b2):
    out, _ = run(dict(x=x, Wg=Wg, bg=bg, W1=W1, b1=b1, W2=W2, b2=b2))
    return out
